# Initial kernel scaffold
#
# kernel.py — DocRE model (segment_reduce) on 8 Trainium2 NeuronCores.
#
# Sharding: data-parallel over (batch, pair-half): core c handles batch c//2,
# pairs [ (c%2)*300, (c%2)*300+300 ) of that batch. Small weights replicated.
#
# Math restructuring vs reference (exact up to float assoc + tf32/bf16 rounding):
#  - e_att drops the 1/cnt normalization (cancels in ht_att row-normalization).
#  - ht_att normalization moved past the rs einsum: rs = (u @ seq) / (u @ 1).
#  - logsumexp computed as log(sum(exp)) (inputs are O(5), no overflow).
#  - proj/cls classifiers folded: logits = bl @ (proj_W.T @ cls_W.T) + cls_b.
#  - all gathers become indirect DMAs (HBM rows) or one-hot matmuls (on-chip).
import numpy as np
import ml_dtypes

import concourse.bass as bass
import concourse.tile as tile
import concourse.mybir as mybir
from concourse import bacc, bass_utils, library_config
from concourse.masks import make_identity

BF16 = ml_dtypes.bfloat16

B, L, H, NH = 4, 1024, 768, 12
NE, M, P = 30, 6, 600
EMB, BS, NCLS = 768, 16, 97
NCORES = 8
PH = P // 2           # 300 pairs per core
NROWS = NH * NE * M   # 2160 gathered attention rows per core
NTILES = (NROWS + 127) // 128          # 17
NROWS_PAD = NTILES * 128               # 2176
MROWS = NE * M                         # 180 mention rows for seq gather
MTILES = (MROWS + 127) // 128          # 2
LT = L // 128                          # 8 l-tiles
HC = H // 128                          # 6 h-chunks
KT = EMB * BS // 128                   # 96 bl K-tiles
F32, BF, I32 = mybir.dt.float32, mybir.dt.bfloat16, mybir.dt.int32
F32R = mybir.dt.float32r
AF = mybir.ActivationFunctionType
ALU = mybir.AluOpType



def _att_segments():
    """Static segmentation of the 17 gathered-row K-tiles by attention head.

    Row r (nh-major) = nh*180 + e*6 + m.  Returns list of
    (tile_idx, nh, is_first_seg_of_nh, is_last_seg_of_nh).  Each segment
    uses the full 128-row K-tile; rows of other heads are zeroed in the
    host-built mask one-hot (matmul base partition must be 0).
    """
    segs = []
    for c in range(NTILES):
        r0, r1 = c * 128, min(c * 128 + 128, NROWS)
        r = r0
        while r < r1:
            nh = r // (NE * M)
            rend = min(r1, (nh + 1) * NE * M)
            segs.append((c, nh, r == nh * NE * M, rend == (nh + 1) * NE * M))
            r = rend
    return segs


SEGS = _att_segments()
NSEG = len(SEGS)  # 28


def build_program(stages=99, reps=1):
    nc = bacc.Bacc("TRN2", target_bir_lowering=False, debug=False,
                   enable_asserts=False, num_devices=NCORES)
    d = {}

    def din(name, shape, dt):
        d[name] = nc.dram_tensor(name, shape, dt, kind="ExternalInput").ap()
        return d[name]

    att = din("att", [NH * L, L], BF)          # attention[b] as rows (bf16)
    seq = din("seq", [L, H], F32)               # sequence_output[b]
    aidx = din("aidx", [128, NROWS_PAD // 16], mybir.dt.int16)
    midx = din("midx", [MTILES, 128], I32)      # mention seq-row ids
    mosum = din("mosum", [NSEG, 128, NE], BF)  # per-seg mask val at entity col
    msel = din("msel", [MTILES, 128, NE], BF)     # (mask>0) at entity col
    ohh = din("ohh", [NE, PH], BF)              # head-entity one-hot per pair
    oht = din("oht", [NE, PH], BF)
    ohsel8 = din("ohsel8", [16, 128, 128], BF)   # bl hs-replication one-hots
    ohsel16 = din("ohsel16", [8, 128, 128], BF)  # bl ts-replication one-hots
    whT = din("whT", [2 * H, EMB], BF)          # head_W.T
    wtT = din("wtT", [2 * H, EMB], BF)          # tail_W.T
    wc = din("wc", [EMB * BS, NCLS], BF)        # proj_W.T @ cls_W.T
    hbb = din("hbb", [NE, EMB], F32)            # head_b broadcast to entities
    tbb = din("tbb", [NE, EMB], F32)
    clsb = din("clsb", [NCLS, 1], F32)
    out = nc.dram_tensor("out", [NCLS, PH], F32, kind="ExternalOutput").ap()

    with tile.TileContext(nc) as tc:
        with (
            nc.allow_low_precision(reason="bf16 head-plane tree reduction"),
            tc.tile_pool(name="persist", bufs=1) as pp,
            tc.tile_pool(name="work", bufs=2) as wp,
        ):
            # ---------- persistent SBUF tensors ----------
            ident = pp.tile([128, 128], F32)
            make_identity(nc, ident[:])
            aidx_sb = pp.tile([128, NROWS_PAD // 16], mybir.dt.int16)
            nc.sync.dma_start(aidx_sb[:], aidx[:])
            attg = pp.tile([128, NTILES, L], BF)
            nc.gpsimd.load_library(library_config.mlp)
            # 4 chunks (512/512/512/640 rows) so stage-1 matmuls can start
            # on chunk 0 while later chunks are still gathering
            for k, nk in enumerate((512, 512, 512, 640)):
                nc.gpsimd.dma_gather(
                    attg[:, k * 4:k * 4 + nk // 128, :], att[:],
                    aidx_sb[:, k * 32:k * 32 + nk // 16], nk, nk, L,
                    single_packet=False)
            midx_sb = pp.tile([128, MTILES], I32)
            nc.sync.dma_start(midx_sb[:], midx[:].rearrange("t p -> p t"))
            mosum_sb = pp.tile([128, NSEG, NE], BF)
            nc.sync.dma_start(mosum_sb[:], mosum[:].rearrange("t p e -> p t e"))
            msel_sb = pp.tile([128, MTILES, NE], BF)
            nc.sync.dma_start(msel_sb[:], msel[:].rearrange("t p e -> p t e"))
            ohh_sb = pp.tile([NE, PH], BF)
            nc.sync.dma_start(ohh_sb[:], ohh[:])
            oht_sb = pp.tile([NE, PH], BF)
            nc.sync.dma_start(oht_sb[:], oht[:])
            oh8_sb = pp.tile([128, 16, 128], BF)
            nc.sync.dma_start(oh8_sb[:], ohsel8[:].rearrange("v p m -> p v m"))
            oh16_sb = pp.tile([128, 8, 128], BF)
            nc.sync.dma_start(oh16_sb[:], ohsel16[:].rearrange("v p m -> p v m"))
            hbb_sb = pp.tile([NE, EMB], F32)
            nc.sync.dma_start(hbb_sb[:], hbb[:])
            tbb_sb = pp.tile([NE, EMB], F32)
            nc.sync.dma_start(tbb_sb[:], tbb[:])
            clsb_sb = pp.tile([NCLS, 1], F32)
            nc.sync.dma_start(clsb_sb[:], clsb[:])
            # weights (bf16): [128, 12, 768] tiled by contraction dim
            whT_sb = pp.tile([128, 12, EMB], BF)
            nc.sync.dma_start(whT_sb[:], whT[:].rearrange("(t p) o -> p t o", p=128))
            wtT_sb = pp.tile([128, 12, EMB], BF)
            nc.sync.dma_start(wtT_sb[:], wtT[:].rearrange("(t p) o -> p t o", p=128))
            wc_sb = pp.tile([128, KT, NCLS], BF)
            nc.sync.dma_start(wc_sb[:], wc[:].rearrange("(t p) n -> p t n", p=128))
            # sequence_output in bf16, l on partitions: [128, 8, 768]
            seq_sb = pp.tile([128, LT, H], BF)
            nc.gpsimd.dma_start(seq_sb[:], seq[:].rearrange("(t p) h -> p t h", p=128))
            onecol = pp.tile([128, 1], BF)
            nc.vector.memset(onecol[:], 1.0)
            onerow = pp.tile([1, 128], BF)
            nc.vector.memset(onerow[:], 1.0)

            # e_att[nh] tiles [30, 1024] bf16 (un-normalized mention sums)
            ea = [pp.tile([NE, L], BF, tag=f"ea{nh}", name=f"ea{nh}")
                  for nh in range(NH)]
            # u tiles [128, 8, 300] bf16 (l on partitions)
            u_sb = pp.tile([128, LT, PH], BF)
            # misc small persistents
            e1h_sb = pp.tile([NE, EMB], BF, tag="e1h")
            e1t_sb = pp.tile([NE, EMB], BF, tag="e1t")
            eembT_sb = pp.tile([128, HC, NE], BF)
            rs_sb = pp.tile([128, HC, PH], BF)
            hs_sb = pp.tile([128, HC, PH], BF, tag="hs")
            ts_sb = pp.tile([128, HC, PH], BF, tag="ts")

            class _Bail(Exception):
                pass

            def bail(n):
                if stages < n:
                    nc.gpsimd.dma_start(
                        out[:], mosum_sb[0:NCLS, 0:10, 0:NE])
                    raise _Bail()

            try:
                for _rep in range(reps):
                    bail(1)
                    # ---------- stage 1: gather attention rows + e_att sums ----------
                    # psum accumulators per head: [30, 1024] f32, <=2 live at a time
                    with (
                        tc.tile_pool(name="ps_att", bufs=2, space="PSUM") as ps_att,
                        tc.tile_pool(name="ps_sm", bufs=1, space="PSUM") as ps_sm,
                    ):
                        psum_nh = {}
                        for si, (c, nh, first, last) in enumerate(_att_segments()):
                            if first:
                                psum_nh[nh] = ps_att.tile([NE, L], F32, tag="pnh",
                                                          name=f"pnh{nh}")
                            for n0 in (0, 512):
                                nc.tensor.matmul(
                                    out=psum_nh[nh][:, n0:n0 + 512],
                                    lhsT=mosum_sb[:, si, :],
                                    rhs=attg[:, c, n0:n0 + 512],
                                    start=first, stop=last)
                            if last:
                                nc.scalar.activation(ea[nh][:], psum_nh[nh][:], AF.Copy)

                        # ------- stage 2a: e_sum = sum(msel * exp(seq rows)) -------
                        psum_ee = ps_sm.tile([NE, EMB], F32, tag="pee")
                        for c in range(MTILES):
                            mg = wp.tile([128, H], F32, tag="mg", bufs=1)
                            nc.gpsimd.indirect_dma_start(
                                out=mg[:], out_offset=None, in_=seq[:],
                                in_offset=bass.IndirectOffsetOnAxis(
                                    ap=midx_sb[:, c:c + 1], axis=0),
                            )
                            ex = wp.tile([128, H], BF, tag="ex")
                            nc.scalar.activation(ex[:], mg[:], AF.Exp)
                            for n0 in (0, 512):
                                nw = min(512, H - n0)
                                nc.tensor.matmul(
                                    out=psum_ee[:, n0:n0 + nw],
                                    lhsT=msel_sb[:, c, :],
                                    rhs=ex[:, n0:n0 + nw],
                                    start=(c == 0), stop=(c == MTILES - 1))
                        eemb = wp.tile([NE, EMB], F32, tag="eemb", bufs=1)
                        # log(x + 1e-37): tiny bias keeps fully-masked entities finite
                        lneps = pp.tile([NE, 1], F32)
                        nc.vector.memset(lneps[:], 1e-37)
                        nc.scalar.activation(eemb[:], psum_ee[:], AF.Ln, bias=lneps[:])

                    bail(2)
                    # ---------- stage 2b: e_embT, E1h/E1t ----------
                    with tc.tile_pool(name="ps_e1", bufs=2, space="PSUM") as ps_e1:
                        # transpose e_emb -> [128, 6, 30] bf16
                        for hc in range(HC):
                            pt = ps_e1.tile([128, NE], F32, tag="ptr")
                            nc.tensor.transpose(pt[:], eemb[:, hc * 128:(hc + 1) * 128],
                                                ident[0:NE, 0:NE])
                            nc.scalar.activation(eembT_sb[:, hc, :], pt[:], AF.Copy)

                        # E1h = e_emb @ Wh1.T + head_b  (and tail analog)
                        for (wT, bb, e1) in ((whT_sb, hbb_sb, e1h_sb),
                                             (wtT_sb, tbb_sb, e1t_sb)):
                            psum_e1 = ps_e1.tile([NE, EMB], F32, tag="pe1")
                            for ic in range(HC):
                                for n0 in (0, 512):
                                    nw = min(512, EMB - n0)
                                    nc.tensor.matmul(
                                        out=psum_e1[:, n0:n0 + nw],
                                        lhsT=eembT_sb[:, ic, :],
                                        rhs=wT[:, ic, n0:n0 + nw],
                                        start=(ic == 0), stop=(ic == HC - 1))
                            nc.vector.tensor_tensor(e1[:], psum_e1[:], bb[:], op=ALU.add)

                    bail(3)
                    # ---------- stage 3: u[l, p] = sum_nh eh*et ----------
                    # psum planes padded to 512 so each matmul output is bank-aligned
                    with tc.tile_pool(name="ps_u", bufs=2, space="PSUM") as ps_u:
                        for lc in range(LT):
                            prod = wp.tile([128, NH, PH], BF, tag="prod", bufs=2)
                            for q in range(6):          # groups of 2 heads
                                hp = ps_u.tile([128, 2, 512], F32, tag="hp")
                                tp_ = ps_u.tile([128, 2, 512], F32, tag="tp")
                                for j in range(2):
                                    nh = q * 2 + j
                                    sl = ea[nh][:, lc * 128:(lc + 1) * 128]
                                    nc.tensor.matmul(hp[:, j, 0:PH], lhsT=sl,
                                                     rhs=ohh_sb[:], start=True, stop=True)
                                    nc.tensor.matmul(tp_[:, j, 0:PH], lhsT=sl,
                                                     rhs=oht_sb[:], start=True, stop=True)
                                tsb = wp.tile([128, 2, PH], BF, tag="tsb", bufs=4)
                                nc.scalar.activation(tsb[:], tp_[:, :, 0:PH], AF.Copy)
                                nc.vector.tensor_tensor(prod[:, q * 2:(q + 1) * 2, :],
                                                        hp[:, :, 0:PH], tsb[:],
                                                        op=ALU.mult)
                            # tree-reduce the 12 head planes (bf16 2x mode)
                            t6 = wp.tile([128, 6, PH], BF, tag="t6", bufs=1)
                            nc.vector.tensor_tensor(t6[:], prod[:, 0:6, :],
                                                    prod[:, 6:12, :], op=ALU.add)
                            t3 = wp.tile([128, 3, PH], BF, tag="t3", bufs=1)
                            nc.vector.tensor_tensor(t3[:], t6[:, 0:3, :], t6[:, 3:6, :],
                                                    op=ALU.add)
                            ta = wp.tile([128, PH], BF, tag="ta", bufs=1)
                            nc.vector.tensor_tensor(ta[:], t3[:, 0, :], t3[:, 1, :],
                                                    op=ALU.add)
                            nc.vector.tensor_tensor(u_sb[:, lc, :], ta[:], t3[:, 2, :],
                                                    op=ALU.add)

                    bail(4)
                    # ---------- stage 4: rs_T = (u @ [seq|1]) / z ----------
                    with tc.tile_pool(name="ps_rs", bufs=1, space="PSUM") as ps_rs:
                        psz = ps_rs.tile([1, PH], F32, tag="psz")
                        for lc in range(LT):
                            nc.tensor.matmul(psz[:], lhsT=onecol[:], rhs=u_sb[:, lc, :],
                                             start=(lc == 0), stop=(lc == LT - 1))
                        rz = wp.tile([1, PH], BF, tag="rz")
                        nc.vector.reciprocal(rz[:], psz[:])
                        pzb = ps_rs.tile([128, PH], F32, tag="pzb")
                        nc.tensor.matmul(pzb[:], lhsT=onerow[:], rhs=rz[:],
                                         start=True, stop=True)
                        zb_sb = wp.tile([128, PH], F32, tag="zb_sb", bufs=1)
                        nc.scalar.activation(zb_sb[:], pzb[:], AF.Copy)
                        for mc in range(HC):
                            prs = ps_rs.tile([128, PH], F32, tag="prs", bufs=3)
                            for lc in range(LT):
                                nc.tensor.matmul(
                                    prs[:],
                                    lhsT=seq_sb[:, lc, mc * 128:(mc + 1) * 128],
                                    rhs=u_sb[:, lc, :],
                                    start=(lc == 0), stop=(lc == LT - 1))
                            nc.vector.tensor_tensor(rs_sb[:, mc, :], prs[:], zb_sb[:],
                                                    op=ALU.mult)

                    bail(5)
                    # ---------- stage 5: hs_T/ts_T = tanh(W2 @ rs + E1[pair]) ----------
                    with tc.tile_pool(name="ps_hs", bufs=1, space="PSUM") as ps_hs:
                        for (wT, e1, oh, dst) in ((whT_sb, e1h_sb, ohh_sb, hs_sb),
                                                  (wtT_sb, e1t_sb, oht_sb, ts_sb)):
                            for oc in range(HC):
                                ph = ps_hs.tile([128, PH], F32, tag="ph", bufs=4)
                                for ic in range(HC):
                                    nc.tensor.matmul(
                                        ph[:],
                                        lhsT=wT[:, HC + ic, oc * 128:(oc + 1) * 128],
                                        rhs=rs_sb[:, ic, :],
                                        start=(ic == 0), stop=False)
                                nc.tensor.matmul(
                                    ph[:], lhsT=e1[:, oc * 128:(oc + 1) * 128], rhs=oh[:],
                                    start=False, stop=True)
                                nc.scalar.activation(dst[:, oc, :], ph[:], AF.Tanh)

                    bail(6)
                    # ---------- stage 6: bl + logits ----------
                    with (
                        tc.tile_pool(name="ps_bl", bufs=2, space="PSUM") as ps_bl,
                        tc.tile_pool(name="ps_lg", bufs=1, space="PSUM") as ps_lg,
                    ):
                        plg = ps_lg.tile([NCLS, PH], F32)
                        for kt in range(KT):
                            kb, half = kt // 2, kt % 2
                            o_h = kb * BS + half * 8          # 8 hs rows
                            o_t = kb * BS                      # 16 ts rows
                            ch, v8 = o_h // 128, (o_h % 128) // 8
                            ct, v16 = o_t // 128, (o_t % 128) // 16
                            hrep = ps_bl.tile([128, PH], F32, tag="hrep", bufs=3)
                            nc.tensor.matmul(
                                hrep[:], lhsT=oh8_sb[:, v8, :],
                                rhs=hs_sb[:, ch, :], start=True, stop=True)
                            trep = ps_bl.tile([128, PH], F32, tag="trep", bufs=3)
                            nc.tensor.matmul(
                                trep[:], lhsT=oh16_sb[:, v16, :],
                                rhs=ts_sb[:, ct, :], start=True, stop=True)
                            blt = wp.tile([128, PH], BF, tag="blt", bufs=4)
                            tr_sb = wp.tile([128, PH], BF, tag="tr_sb", bufs=4)
                            nc.scalar.activation(tr_sb[:], trep[:], AF.Copy)
                            nc.vector.tensor_tensor(blt[:], hrep[:], tr_sb[:],
                                                    op=ALU.mult)
                            nc.tensor.matmul(plg[:], lhsT=wc_sb[:, kt, :], rhs=blt[:],
                                             start=(kt == 0), stop=(kt == KT - 1))
                        og = wp.tile([NCLS, PH], F32, tag="og", bufs=1)
                        nc.scalar.activation(og[:], plg[:], AF.Identity, bias=clsb_sb[:])
                        nc.sync.dma_start(out[:], og[:])
            except _Bail:
                pass

    nc.compile()
    return nc


_PROG = None


def _host_prep(inputs):
    """Build the 8 per-core input maps from full inputs."""
    seqf = np.asarray(inputs["sequence_output"], np.float32)
    attf = np.asarray(inputs["attention"], np.float32)
    mask = np.asarray(inputs["mention_mask"], np.float32)
    midx_full = np.asarray(inputs["mention_idx"]).astype(np.int64)
    hts = np.asarray(inputs["hts"]).astype(np.int64)
    head_W = np.asarray(inputs["head_W"], np.float32)
    tail_W = np.asarray(inputs["tail_W"], np.float32)
    head_b = np.asarray(inputs["head_b"], np.float32)
    tail_b = np.asarray(inputs["tail_b"], np.float32)
    proj_W = np.asarray(inputs["proj_W"], np.float32)
    cls_W = np.asarray(inputs["cls_W"], np.float32)
    cls_b = np.asarray(inputs["cls_b"], np.float32)

    wc = (proj_W.T @ cls_W.T).astype(BF16)                     # [12288, 97]
    whT = np.ascontiguousarray(head_W.T).astype(BF16)          # [1536, 768]
    wtT = np.ascontiguousarray(tail_W.T).astype(BF16)
    hbb = np.broadcast_to(head_b, (NE, EMB)).astype(np.float32).copy()
    tbb = np.broadcast_to(tail_b, (NE, EMB)).astype(np.float32).copy()
    clsb = cls_b.reshape(NCLS, 1).copy()
    pp_, mm_ = np.meshgrid(np.arange(128), np.arange(128), indexing="ij")
    ohsel8 = np.stack([(pp_ == 8 * v + mm_ // 16) for v in range(16)]
                      ).astype(BF16)
    ohsel16 = np.stack([(pp_ == 16 * v + mm_ % 16) for v in range(8)]
                       ).astype(BF16)

    in_maps = []
    for c in range(NCORES):
        b, half = c // 2, c % 2
        p0 = half * PH
        # attention-row gather ids: r = nh*180 + e*6 + m -> nh*1024 + idx
        rows = (np.arange(NH)[:, None] * L +
                midx_full[b].reshape(1, -1)).reshape(-1)       # [2160]
        aidx = np.zeros(NROWS_PAD, np.int16)
        aidx[:NROWS] = rows.astype(np.int16)
        aidx16 = np.tile(aidx.reshape(-1, 16).T, (8, 1)).astype(np.int16)
        # mosum: per-segment [128, NE] with mask value at entity column,
        # rows outside the segment's head zeroed
        ecol = np.tile(np.repeat(np.arange(NE), M), NH)        # [2160]
        mval = np.tile(mask[b].reshape(-1), NH)                # [2160]
        dense = np.zeros((NROWS_PAD, NE), np.float32)
        dense[np.arange(NROWS), ecol] = mval
        rownh = np.full(NROWS_PAD, -1, np.int64)
        rownh[:NROWS] = np.arange(NROWS) // (NE * M)
        mosum = np.zeros((NSEG, 128, NE), BF16)
        for si, (c, nh, _f, _l) in enumerate(SEGS):
            rows = slice(c * 128, c * 128 + 128)
            mosum[si] = dense[rows] * (rownh[rows] == nh)[:, None]
        # mention seq gather
        midx = np.zeros(MTILES * 128, np.int32)
        midx[:MROWS] = midx_full[b].reshape(-1)
        msel = np.zeros((MTILES * 128, NE), BF16)
        msel[np.arange(MROWS), np.repeat(np.arange(NE), M)] = \
            (mask[b].reshape(-1) > 0).astype(np.float32)
        # pair one-hots
        hh = hts[b, p0:p0 + PH, 0]
        tt = hts[b, p0:p0 + PH, 1]
        ohh = np.zeros((NE, PH), BF16)
        ohh[hh, np.arange(PH)] = 1
        oht = np.zeros((NE, PH), BF16)
        oht[tt, np.arange(PH)] = 1
        in_maps.append({
            "att": attf[b].reshape(NH * L, L).astype(BF16),
            "seq": seqf[b],
            "aidx": aidx16,
            "midx": midx.reshape(MTILES, 128),
            "mosum": mosum,
            "msel": msel.reshape(MTILES, 128, NE),
            "ohh": ohh, "oht": oht, "ohsel8": ohsel8, "ohsel16": ohsel16,
            "whT": whT, "wtT": wtT, "wc": wc,
            "hbb": hbb, "tbb": tbb, "clsb": clsb,
        })
    return in_maps


def kernel(**inputs):
    global _PROG
    if _PROG is None:
        _PROG = build_program()
    in_maps = _host_prep(inputs)
    res = bass_utils.run_bass_kernel_spmd(
        _PROG, in_maps, core_ids=list(range(NCORES)))
    logits = np.zeros((B, P, NCLS), np.float32)
    for c in range(NCORES):
        b, half = c // 2, c % 2
        logits[b, half * PH:(half + 1) * PH, :] = res.results[c]["out"].T
    return logits



# revision 19
# speedup vs baseline: 1.2229x; 1.2229x over previous
# kernel.py — DocRE model (segment_reduce) on 8 Trainium2 NeuronCores.
#
# Sharding: data-parallel over (batch, pair-half): core c handles batch c//2,
# pairs [ (c%2)*300, (c%2)*300+300 ) of that batch. Small weights replicated.
#
# Math restructuring vs reference (exact up to float assoc + tf32/bf16 rounding):
#  - e_att drops the 1/cnt normalization (cancels in ht_att row-normalization).
#  - ht_att normalization moved past the rs einsum: rs = (u @ seq) / (u @ 1).
#  - logsumexp computed as log(sum(exp)) (inputs are O(5), no overflow).
#  - proj/cls classifiers folded: logits = bl @ (proj_W.T @ cls_W.T) + cls_b.
#  - all gathers become indirect DMAs (HBM rows) or one-hot matmuls (on-chip).
#
# Engine budget (cost-model): PE streams one output row (<=512 wide) per
# 0.417ns; DVE pays 1.04ns/el on any PSUM operand, 0.52 on packed bf16 SBUF;
# Act ~0.83ns/el + ~290ns fixed; GPSIMD has no PSUM port. The layout below
# keeps PE saturated (it is the bottleneck) and keeps DVE ops large.
import numpy as np
import ml_dtypes

import concourse.bass as bass
import concourse.tile as tile
import concourse.mybir as mybir
from concourse import bacc, bass_utils, library_config
from concourse.masks import make_identity

BF16 = ml_dtypes.bfloat16

B, L, H, NH = 4, 1024, 768, 12
NE, M, P = 30, 6, 600
EMB, BS, NCLS = 768, 16, 97
NCORES = 8
PH = P // 2           # 300 pairs per core
NROWS = NH * NE * M   # 2160 gathered attention rows per core
NTILES = (NROWS + 127) // 128          # 17
NROWS_PAD = NTILES * 128               # 2176
MROWS = NE * M                         # 180 mention rows for seq gather
MTILES = (MROWS + 127) // 128          # 2
LT = L // 128                          # 8 l-tiles
HC = H // 128                          # 6 h-chunks
KT = EMB * BS // 128                   # 96 bl K-tiles
KB = KT // 2                           # 48 c-blocks (16 rows of hs/ts each)
F32, BF, I32 = mybir.dt.float32, mybir.dt.bfloat16, mybir.dt.int32
AF = mybir.ActivationFunctionType
ALU = mybir.AluOpType


def _att_segments():
    """Static segmentation of the 17 gathered-row K-tiles by attention head.

    Row r (nh-major) = nh*180 + e*6 + m.  Returns list of
    (tile_idx, nh, is_first_seg_of_nh, is_last_seg_of_nh).  Each segment
    uses the full 128-row K-tile; rows of other heads are zeroed in the
    host-built mask one-hot (matmul base partition must be 0).
    """
    segs = []
    for c in range(NTILES):
        r0, r1 = c * 128, min(c * 128 + 128, NROWS)
        r = r0
        while r < r1:
            nh = r // (NE * M)
            rend = min(r1, (nh + 1) * NE * M)
            segs.append((c, nh, r == nh * NE * M, rend == (nh + 1) * NE * M))
            r = rend
    return segs


SEGS = _att_segments()
NSEG = len(SEGS)  # 28


def build_program(stages=99, reps=1):
    nc = bacc.Bacc("TRN2", target_bir_lowering=False, debug=False,
                   enable_asserts=False, num_devices=NCORES)
    d = {}

    def din(name, shape, dt):
        d[name] = nc.dram_tensor(name, shape, dt, kind="ExternalInput").ap()
        return d[name]

    att = din("att", [NH * L, L], BF)          # attention[b] as rows (bf16)
    seq = din("seq", [L, H], F32)               # sequence_output[b] (row gather)
    seqb = din("seqb", [L, H], BF)              # same, bf16 (rs einsum lhsT)
    aidx = din("aidx", [128, NROWS_PAD // 16], mybir.dt.int16)
    midx = din("midx", [MTILES, 128], I32)      # mention seq-row ids
    mosum = din("mosum", [NSEG, 128, NE], BF)  # per-seg mask val at entity col
    msel = din("msel", [MTILES, 128, NE], BF)     # (mask>0) at entity col
    ohh = din("ohh", [NE, PH], BF)              # head-entity one-hot per pair
    oht = din("oht", [NE, PH], BF)
    ohsel8 = din("ohsel8", [16, 128, 128], BF)   # bl hs-replication one-hots
    ohsel16 = din("ohsel16", [8, 128, 128], BF)  # bl ts-replication one-hots
    whT = din("whT", [2 * H, EMB], BF)          # head_W.T
    wtT = din("wtT", [2 * H, EMB], BF)          # tail_W.T
    wc = din("wc", [EMB * BS, NCLS], BF)        # proj_W.T @ cls_W.T
    hbb = din("hbb", [NE, EMB], F32)            # head_b broadcast to entities
    tbb = din("tbb", [NE, EMB], F32)
    clsb = din("clsb", [NCLS, 1], F32)
    out = nc.dram_tensor("out", [NCLS, PH], F32, kind="ExternalOutput").ap()

    with tile.TileContext(nc) as tc:
        with (
            nc.allow_low_precision(reason="bf16 head-plane tree reduction"),
            tc.tile_pool(name="persist", bufs=1) as pp,
            tc.tile_pool(name="work", bufs=2) as wp,
        ):
            # ---------- persistent SBUF tensors ----------
            ident = pp.tile([128, 128], F32)
            make_identity(nc, ident[:])
            # DMA order == need order: tiny metas, then the attention gather
            # (gates fused S1+S3), then everything needed from stage 4 on,
            # which streams underneath the fused compute phase.
            aidx_sb = pp.tile([128, NROWS_PAD // 16], mybir.dt.int16)
            nc.sync.dma_start(aidx_sb[:], aidx[:])
            midx_sb = pp.tile([128, MTILES], I32)
            nc.sync.dma_start(midx_sb[:], midx[:].rearrange("t p -> p t"))
            msel_sb = pp.tile([128, MTILES, NE], BF)
            nc.sync.dma_start(msel_sb[:], msel[:].rearrange("t p e -> p t e"))
            mosum_sb = pp.tile([128, NSEG, NE], BF)
            nc.sync.dma_start(mosum_sb[:], mosum[:].rearrange("t p e -> p t e"))
            ohh_sb = pp.tile([NE, PH], BF)
            nc.sync.dma_start(ohh_sb[:], ohh[:])
            oht_sb = pp.tile([NE, PH], BF)
            nc.sync.dma_start(oht_sb[:], oht[:])
            attg = pp.tile([128, NTILES, L], BF)
            nc.gpsimd.load_library(library_config.mlp)
            # 4 chunks (512/512/512/640 rows) so stage-1 matmuls can start
            # on chunk 0 while later chunks are still gathering
            for k, nk in enumerate((512, 512, 512, 640)):
                nc.gpsimd.dma_gather(
                    attg[:, k * 4:k * 4 + nk // 128, :], att[:],
                    aidx_sb[:, k * 32:k * 32 + nk // 16], nk, nk, L,
                    single_packet=False)
            # mention-row gather + exp now: ex_sb sits ready for stage 2,
            # which is deferred past the fused S1+S3 phase
            ex_sb = pp.tile([128, MTILES, H], BF)
            for c in range(MTILES):
                mg = wp.tile([128, H], F32, tag="mg", bufs=2)
                nc.gpsimd.indirect_dma_start(
                    out=mg[:], out_offset=None, in_=seq[:],
                    in_offset=bass.IndirectOffsetOnAxis(
                        ap=midx_sb[:, c:c + 1], axis=0),
                )
                nc.scalar.activation(ex_sb[:, c, :], mg[:], AF.Exp)
            # heavy loads ride the gpsimd SWDGE queue BEHIND the attention
            # gather: strict in-queue order gives the gather (which gates the
            # fused S1+S3 phase) full DMA bandwidth; these stream under compute
            seq_sb = pp.tile([128, LT, H], BF)
            nc.gpsimd.dma_start(seq_sb[:], seqb[:].rearrange("(t p) h -> p t h", p=128))
            whT_sb = pp.tile([128, 12, EMB], BF)
            nc.gpsimd.dma_start(whT_sb[:], whT[:].rearrange("(t p) o -> p t o", p=128))
            wtT_sb = pp.tile([128, 12, EMB], BF)
            nc.gpsimd.dma_start(wtT_sb[:], wtT[:].rearrange("(t p) o -> p t o", p=128))
            hbb_sb = pp.tile([NE, EMB], F32)
            nc.sync.dma_start(hbb_sb[:], hbb[:])
            tbb_sb = pp.tile([NE, EMB], F32)
            nc.sync.dma_start(tbb_sb[:], tbb[:])
            clsb_sb = pp.tile([NCLS, 1], F32)
            nc.sync.dma_start(clsb_sb[:], clsb[:])
            oh8_sb = pp.tile([128, 16, 128], BF)
            nc.gpsimd.dma_start(oh8_sb[:], ohsel8[:].rearrange("v p m -> p v m"))
            oh16_sb = pp.tile([128, 8, 128], BF)
            nc.gpsimd.dma_start(oh16_sb[:], ohsel16[:].rearrange("v p m -> p v m"))
            wc_sb = pp.tile([128, KT, NCLS], BF)
            nc.gpsimd.dma_start(wc_sb[:], wc[:].rearrange("(t p) n -> p t n", p=128))
            onecol = pp.tile([128, 1], BF)
            nc.vector.memset(onecol[:], 1.0)
            onerow = pp.tile([1, 128], BF)
            nc.vector.memset(onerow[:], 1.0)

            # e_att[nh] tiles [30, 1024] bf16 (un-normalized mention sums)
            ea = [pp.tile([NE, L], BF, tag=f"ea{nh}", name=f"ea{nh}")
                  for nh in range(NH)]
            # u tiles [128, 8, 300] bf16 (l on partitions)
            u_sb = pp.tile([128, LT, PH], BF)
            # misc small persistents
            e1h_sb = pp.tile([NE, EMB], BF, tag="e1h")
            e1t_sb = pp.tile([NE, EMB], BF, tag="e1t")
            eembT_sb = pp.tile([128, HC, NE], BF)
            rs_sb = pp.tile([128, HC, PH], BF)
            hs_sb = pp.tile([128, HC, PH], BF, tag="hs")
            ts_sb = pp.tile([128, HC, PH], BF, tag="ts")

            class _Bail(Exception):
                pass

            def bail(n):
                if stages < n:
                    nc.gpsimd.dma_start(
                        out[:], mosum_sb[0:NCLS, 0:10, 0:NE])
                    raise _Bail()

            # u accumulator: [128, lt, 2, 300] bf16, two half-planes folded
            # into u_sb at stage-4 entry
            u2_sb = pp.tile([128, LT, 2, PH], BF)

            try:
                for _rep in range(reps):
                    bail(1)
                    # ---- fused stages 1+3, software-pipelined by head pair:
                    # stage-1 segment matmuls for pair q+1 run on PE while
                    # stage-3 expansion products for pair q drain through
                    # Act (tail-side evac) and DVE (mult + accumulate).
                    # PSUM: pnh 2 banks + hp 2x2 + tp 2x1 = 8 banks. ----
                    with tc.tile_pool(name="ps_f", bufs=1, space="PSUM") as ps_f:
                        segs_of = {}
                        for si, (c, nh, first, last) in enumerate(SEGS):
                            segs_of.setdefault(nh, []).append((si, c, first, last))

                        def emit_s1_pair(q):
                            for nh in (2 * q, 2 * q + 1):
                                pnh = ps_f.tile([NE, L], F32, tag="pnh", bufs=1,
                                                name=f"pnh{nh}")
                                for si, c, first, last in segs_of[nh]:
                                    for n0 in (0, 512):
                                        nc.tensor.matmul(
                                            out=pnh[:, n0:n0 + 512],
                                            lhsT=mosum_sb[:, si, :],
                                            rhs=attg[:, c, n0:n0 + 512],
                                            start=first, stop=last)
                                nc.scalar.activation(ea[nh][:], pnh[:], AF.Copy)

                        def emit_s3_pair(q):
                            for lc in range(LT):
                                hp = ps_f.tile([128, 2, 512], F32, tag="hp", bufs=2)
                                tp = ps_f.tile([128, 2, 512], F32, tag="tp", bufs=1)
                                for j in range(2):
                                    sl = ea[q * 2 + j][:, lc * 128:(lc + 1) * 128]
                                    nc.tensor.matmul(tp[:, j, 0:PH], lhsT=sl,
                                                     rhs=oht_sb[:], start=True,
                                                     stop=True)
                                    nc.tensor.matmul(hp[:, j, 0:PH], lhsT=sl,
                                                     rhs=ohh_sb[:], start=True,
                                                     stop=True)
                                # one PSUM operand max per DVE op: evacuate
                                # the tail side on Act, multiply on DVE
                                tq = wp.tile([128, 2, PH], BF, tag="tq", bufs=3)
                                nc.scalar.activation(tq[:], tp[:, :, 0:PH], AF.Copy)
                                if q == 0:
                                    nc.vector.tensor_tensor(
                                        u2_sb[:, lc, :, :], hp[:, :, 0:PH], tq[:],
                                        op=ALU.mult)
                                else:
                                    pr = wp.tile([128, 2, PH], BF, tag="pr", bufs=3)
                                    nc.vector.tensor_tensor(
                                        pr[:], hp[:, :, 0:PH], tq[:], op=ALU.mult)
                                    # SBUF-only accumulate: odd l-tiles ride
                                    # the otherwise-idle GPSIMD engine
                                    eng = nc.gpsimd if lc % 2 else nc.vector
                                    eng.tensor_tensor(
                                        u2_sb[:, lc, :, :], u2_sb[:, lc, :, :],
                                        pr[:], op=ALU.add)

                        # s3(q) first, s1(q+1) second: the in-order PE queue
                        # serves stage-3's expansion matmuls (which gate DVE)
                        # before the next pair's segment sums, and the Act
                        # queue serves tq copies before the next ea evacs
                        emit_s1_pair(0)
                        for q in range(6):
                            emit_s3_pair(q)
                            if q + 1 < 6:
                                emit_s1_pair(q + 1)

                    bail(4)
                    # ---- stage 2 (deferred): e_emb + E1h/E1t; overlaps
                    # with the stage-4 window below ----
                    with tc.tile_pool(name="ps_sm", bufs=1, space="PSUM") as ps_sm2, \
                         tc.tile_pool(name="ps_rs", bufs=1, space="PSUM") as ps_rs:
                        psum_ee = ps_sm2.tile([NE, EMB], F32, tag="pee")
                        for c in range(MTILES):
                            nc.tensor.matmul(
                                out=psum_ee[:, 0:512],
                                lhsT=msel_sb[:, c, :], rhs=ex_sb[:, c, 0:512],
                                start=(c == 0), stop=(c == MTILES - 1))
                            nc.tensor.matmul(
                                out=psum_ee[:, 512:H],
                                lhsT=msel_sb[:, c, :], rhs=ex_sb[:, c, 512:H],
                                start=(c == 0), stop=(c == MTILES - 1))
                        eemb = wp.tile([NE, EMB], F32, tag="eemb", bufs=1)
                        # log(x + 1e-37): tiny bias keeps fully-masked entities finite
                        lneps = pp.tile([NE, 1], F32)
                        nc.vector.memset(lneps[:], 1e-37)
                        nc.scalar.activation(eemb[:], psum_ee[:], AF.Ln, bias=lneps[:])
                        for hc in range(HC):
                            pt = ps_sm2.tile([128, NE], F32, tag="ptr2", bufs=1)
                            nc.tensor.transpose(pt[:], eemb[:, hc * 128:(hc + 1) * 128],
                                                ident[0:NE, 0:NE])
                            nc.scalar.activation(eembT_sb[:, hc, :], pt[:], AF.Copy)
                        for (wT, bb, e1) in ((whT_sb, hbb_sb, e1h_sb),
                                             (wtT_sb, tbb_sb, e1t_sb)):
                            psum_e1 = ps_sm2.tile([NE, EMB], F32, tag="pee", bufs=1)
                            for ic in range(HC):
                                for n0 in (0, 512):
                                    nw = min(512, EMB - n0)
                                    nc.tensor.matmul(
                                        out=psum_e1[:, n0:n0 + nw],
                                        lhsT=eembT_sb[:, ic, :],
                                        rhs=wT[:, ic, n0:n0 + nw],
                                        start=(ic == 0), stop=(ic == HC - 1))
                            nc.vector.tensor_tensor(e1[:], psum_e1[:], bb[:], op=ALU.add)

                        # ---- stage 4: rs_T = (u @ [seq|1]) / z ----
                        for lc in range(LT):
                            nc.vector.tensor_tensor(
                                u_sb[:, lc, :], u2_sb[:, lc, 0, :],
                                u2_sb[:, lc, 1, :], op=ALU.add)
                        psz = ps_rs.tile([1, PH], F32, tag="psz")
                        for lc in range(LT):
                            nc.tensor.matmul(psz[:], lhsT=onecol[:], rhs=u_sb[:, lc, :],
                                             start=(lc == 0), stop=(lc == LT - 1))
                        rz = wp.tile([1, PH], BF, tag="rz")
                        nc.vector.reciprocal(rz[:], psz[:])
                        pzb = ps_rs.tile([128, PH], F32, tag="pzb")
                        nc.tensor.matmul(pzb[:], lhsT=onerow[:], rhs=rz[:],
                                         start=True, stop=True)
                        zb_sb = wp.tile([128, PH], F32, tag="zb_sb", bufs=1)
                        nc.scalar.activation(zb_sb[:], pzb[:], AF.Copy)
                        for mc in range(HC):
                            prs = ps_rs.tile([128, PH], F32, tag="prs", bufs=3)
                            for lc in range(LT):
                                nc.tensor.matmul(
                                    prs[:],
                                    lhsT=seq_sb[:, lc, mc * 128:(mc + 1) * 128],
                                    rhs=u_sb[:, lc, :],
                                    start=(lc == 0), stop=(lc == LT - 1))
                            nc.vector.tensor_tensor(rs_sb[:, mc, :], prs[:], zb_sb[:],
                                                    op=ALU.mult)

                    bail(5)
                    # ---- stage 5+6 interleaved: extractors feed the block-
                    # bilinear classifier as soon as each 128-chunk is ready ----
                    with (
                        tc.tile_pool(name="ps_hs", bufs=1, space="PSUM") as ps_hs,
                        tc.tile_pool(name="ps_bl", bufs=2, space="PSUM") as ps_bl,
                        tc.tile_pool(name="ps_lg", bufs=1, space="PSUM") as ps_lg,
                    ):
                        plg = ps_lg.tile([NCLS, PH], F32)
                        for oc in range(HC):
                            # hs/ts chunk oc = tanh(W2 @ rs + E1[pair])
                            for (wT, e1, oh, dst) in ((whT_sb, e1h_sb, ohh_sb, hs_sb),
                                                      (wtT_sb, e1t_sb, oht_sb, ts_sb)):
                                ph = ps_hs.tile([128, PH], F32, tag="ph", bufs=2)
                                for ic in range(HC):
                                    nc.tensor.matmul(
                                        ph[:],
                                        lhsT=wT[:, HC + ic, oc * 128:(oc + 1) * 128],
                                        rhs=rs_sb[:, ic, :],
                                        start=(ic == 0), stop=False)
                                nc.tensor.matmul(
                                    ph[:], lhsT=e1[:, oc * 128:(oc + 1) * 128],
                                    rhs=oh[:], start=False, stop=True)
                                nc.scalar.activation(dst[:, oc, :], ph[:], AF.Tanh)

                            if stages < 6:
                                continue
                            # classifier blocks kb using hs/ts chunk oc:
                            # trep computed once per kb, reused by both halves
                            for kb in range(oc * 8, oc * 8 + 8):
                                v16 = kb % 8
                                ptr = ps_bl.tile([128, 512], F32, tag="ptr",
                                                 bufs=1)
                                nc.tensor.matmul(
                                    ptr[:, 0:PH], lhsT=oh16_sb[:, v16, :],
                                    rhs=ts_sb[:, oc, :], start=True, stop=True)
                                # trep used by both halves: evacuate once (Act)
                                trs = wp.tile([128, PH], BF, tag="trs", bufs=2)
                                nc.scalar.activation(trs[:], ptr[:, 0:PH],
                                                     AF.Copy)
                                phr = ps_bl.tile([128, 2, 512], F32, tag="phr")
                                for half in (0, 1):
                                    v8 = (kb % 8) * 2 + half
                                    nc.tensor.matmul(
                                        phr[:, half, 0:PH], lhsT=oh8_sb[:, v8, :],
                                        rhs=hs_sb[:, oc, :], start=True, stop=True)
                                blt = wp.tile([128, 2, PH], BF, tag="blt", bufs=2)
                                nc.vector.tensor_tensor(
                                    blt[:], phr[:, :, 0:PH],
                                    trs[:].unsqueeze(1).broadcast_to(
                                        (128, 2, PH)),
                                    op=ALU.mult)
                                for half in (0, 1):
                                    kt = kb * 2 + half
                                    nc.tensor.matmul(
                                        plg[:], lhsT=wc_sb[:, kt, :],
                                        rhs=blt[:, half, :],
                                        start=(kt == 0), stop=(kt == KT - 1))
                        bail(6)
                        og = wp.tile([NCLS, PH], F32, tag="og", bufs=1)
                        nc.scalar.activation(og[:], plg[:], AF.Identity,
                                             bias=clsb_sb[:])
                        nc.sync.dma_start(out[:], og[:])
            except _Bail:
                pass

    nc.compile()
    return nc


_PROG = None
TRACE = False          # set True (e.g. from test.py) to profile the run
LAST_RES = None        # BassKernelResults of the last kernel() call


def _host_prep(inputs):
    """Build the 8 per-core input maps from full inputs."""
    seqf = np.asarray(inputs["sequence_output"], np.float32)
    attf = np.asarray(inputs["attention"], np.float32)
    mask = np.asarray(inputs["mention_mask"], np.float32)
    midx_full = np.asarray(inputs["mention_idx"]).astype(np.int64)
    hts = np.asarray(inputs["hts"]).astype(np.int64)
    head_W = np.asarray(inputs["head_W"], np.float32)
    tail_W = np.asarray(inputs["tail_W"], np.float32)
    head_b = np.asarray(inputs["head_b"], np.float32)
    tail_b = np.asarray(inputs["tail_b"], np.float32)
    proj_W = np.asarray(inputs["proj_W"], np.float32)
    cls_W = np.asarray(inputs["cls_W"], np.float32)
    cls_b = np.asarray(inputs["cls_b"], np.float32)

    wc = (proj_W.T @ cls_W.T).astype(BF16)                     # [12288, 97]
    whT = np.ascontiguousarray(head_W.T).astype(BF16)          # [1536, 768]
    wtT = np.ascontiguousarray(tail_W.T).astype(BF16)
    hbb = np.broadcast_to(head_b, (NE, EMB)).astype(np.float32).copy()
    tbb = np.broadcast_to(tail_b, (NE, EMB)).astype(np.float32).copy()
    clsb = cls_b.reshape(NCLS, 1).copy()
    pp_, mm_ = np.meshgrid(np.arange(128), np.arange(128), indexing="ij")
    ohsel8 = np.stack([(pp_ == 8 * v + mm_ // 16) for v in range(16)]
                      ).astype(BF16)
    ohsel16 = np.stack([(pp_ == 16 * v + mm_ % 16) for v in range(8)]
                       ).astype(BF16)

    in_maps = []
    for c in range(NCORES):
        b, half = c // 2, c % 2
        p0 = half * PH
        # attention-row gather ids: r = nh*180 + e*6 + m -> nh*1024 + idx
        rows = (np.arange(NH)[:, None] * L +
                midx_full[b].reshape(1, -1)).reshape(-1)       # [2160]
        aidx = np.zeros(NROWS_PAD, np.int16)
        aidx[:NROWS] = rows.astype(np.int16)
        aidx16 = np.tile(aidx.reshape(-1, 16).T, (8, 1)).astype(np.int16)
        # mosum: per-segment [128, NE] with mask value at entity column,
        # rows outside the segment's head zeroed
        ecol = np.tile(np.repeat(np.arange(NE), M), NH)        # [2160]
        mval = np.tile(mask[b].reshape(-1), NH)                # [2160]
        dense = np.zeros((NROWS_PAD, NE), np.float32)
        dense[np.arange(NROWS), ecol] = mval
        rownh = np.full(NROWS_PAD, -1, np.int64)
        rownh[:NROWS] = np.arange(NROWS) // (NE * M)
        mosum = np.zeros((NSEG, 128, NE), BF16)
        for si, (ci, nh, _f, _l) in enumerate(SEGS):
            rws = slice(ci * 128, ci * 128 + 128)
            mosum[si] = dense[rws] * (rownh[rws] == nh)[:, None]
        # mention seq gather
        midx = np.zeros(MTILES * 128, np.int32)
        midx[:MROWS] = midx_full[b].reshape(-1)
        msel = np.zeros((MTILES * 128, NE), BF16)
        msel[np.arange(MROWS), np.repeat(np.arange(NE), M)] = \
            (mask[b].reshape(-1) > 0).astype(np.float32)
        # pair one-hots
        hh = hts[b, p0:p0 + PH, 0]
        tt = hts[b, p0:p0 + PH, 1]
        ohh = np.zeros((NE, PH), BF16)
        ohh[hh, np.arange(PH)] = 1
        oht = np.zeros((NE, PH), BF16)
        oht[tt, np.arange(PH)] = 1
        in_maps.append({
            "att": attf[b].reshape(NH * L, L).astype(BF16),
            "seq": seqf[b],
            "seqb": seqf[b].astype(BF16),
            "aidx": aidx16,
            "midx": midx.reshape(MTILES, 128),
            "mosum": mosum,
            "msel": msel.reshape(MTILES, 128, NE),
            "ohh": ohh, "oht": oht, "ohsel8": ohsel8, "ohsel16": ohsel16,
            "whT": whT, "wtT": wtT, "wc": wc,
            "hbb": hbb, "tbb": tbb, "clsb": clsb,
        })
    return in_maps


def kernel(**inputs):
    global _PROG, LAST_RES
    if _PROG is None:
        _PROG = build_program()
    in_maps = _host_prep(inputs)
    res = bass_utils.run_bass_kernel_spmd(
        _PROG, in_maps, core_ids=list(range(NCORES)), trace=TRACE)
    LAST_RES = res
    logits = np.zeros((B, P, NCLS), np.float32)
    for c in range(NCORES):
        b, half = c // 2, c % 2
        logits[b, half * PH:(half + 1) * PH, :] = res.results[c]["out"].T
    return logits


# revision 20
# speedup vs baseline: 1.3456x; 1.1003x over previous
# kernel.py — DocRE model (segment_reduce) on 8 Trainium2 NeuronCores.
#
# Sharding: data-parallel over (batch, pair-half): core c handles batch c//2,
# pairs [ (c%2)*300, (c%2)*300+300 ) of that batch. Small weights replicated.
#
# Math restructuring vs reference (exact up to float assoc + tf32/bf16 rounding):
#  - e_att drops the 1/cnt normalization (cancels in ht_att row-normalization).
#  - ht_att normalization moved past the rs einsum: rs = (u @ seq) / (u @ 1).
#  - logsumexp computed as log(sum(exp)) (inputs are O(5), no overflow).
#  - proj/cls classifiers folded: logits = bl @ (proj_W.T @ cls_W.T) + cls_b.
#  - all gathers become indirect DMAs (HBM rows) or one-hot matmuls (on-chip).
#
# Engine budget (cost-model): PE streams one output row (<=512 wide) per
# 0.417ns; DVE pays 1.04ns/el on any PSUM operand, 0.52 on packed bf16 SBUF;
# Act ~0.83ns/el + ~290ns fixed; GPSIMD has no PSUM port. The layout below
# keeps PE saturated (it is the bottleneck) and keeps DVE ops large.
import numpy as np
import ml_dtypes

import concourse.bass as bass
import concourse.tile as tile
import concourse.mybir as mybir
from concourse import bacc, bass_utils, library_config
from concourse.masks import make_identity

BF16 = ml_dtypes.bfloat16

B, L, H, NH = 4, 1024, 768, 12
NE, M, P = 30, 6, 600
EMB, BS, NCLS = 768, 16, 97
NCORES = 8
PH = P // 2           # 300 pairs per core
NROWS = NH * NE * M   # 2160 gathered attention rows per core
NTILES = (NROWS + 127) // 128          # 17
NROWS_PAD = NTILES * 128               # 2176
MROWS = NE * M                         # 180 mention rows for seq gather
MTILES = (MROWS + 127) // 128          # 2
LT = L // 128                          # 8 l-tiles
HC = H // 128                          # 6 h-chunks
KT = EMB * BS // 128                   # 96 bl K-tiles
KB = KT // 2                           # 48 c-blocks (16 rows of hs/ts each)
F32, BF, I32 = mybir.dt.float32, mybir.dt.bfloat16, mybir.dt.int32
AF = mybir.ActivationFunctionType
ALU = mybir.AluOpType


def _att_segments():
    """Static segmentation of the 17 gathered-row K-tiles by attention head.

    Row r (nh-major) = nh*180 + e*6 + m.  Returns list of
    (tile_idx, nh, is_first_seg_of_nh, is_last_seg_of_nh).  Each segment
    uses the full 128-row K-tile; rows of other heads are zeroed in the
    host-built mask one-hot (matmul base partition must be 0).
    """
    segs = []
    for c in range(NTILES):
        r0, r1 = c * 128, min(c * 128 + 128, NROWS)
        r = r0
        while r < r1:
            nh = r // (NE * M)
            rend = min(r1, (nh + 1) * NE * M)
            segs.append((c, nh, r == nh * NE * M, rend == (nh + 1) * NE * M))
            r = rend
    return segs


SEGS = _att_segments()
NSEG = len(SEGS)  # 28


def build_program(stages=99, reps=1):
    nc = bacc.Bacc("TRN2", target_bir_lowering=False, debug=False,
                   enable_asserts=False, num_devices=NCORES)
    d = {}

    def din(name, shape, dt):
        d[name] = nc.dram_tensor(name, shape, dt, kind="ExternalInput").ap()
        return d[name]

    att = din("att", [NH * L, L], BF)          # attention[b] as rows (bf16)
    seq = din("seq", [L, H], F32)               # sequence_output[b] (row gather)
    seqb = din("seqb", [L, H], BF)              # same, bf16 (rs einsum lhsT)
    aidx = din("aidx", [128, NROWS_PAD // 16], mybir.dt.int16)
    midx = din("midx", [MTILES, 128], I32)      # mention seq-row ids
    mosum = din("mosum", [NSEG, 128, NE], BF)  # per-seg mask val at entity col
    msel = din("msel", [MTILES, 128, NE], BF)     # (mask>0) at entity col
    ohh = din("ohh", [NE, PH], BF)              # head-entity one-hot per pair
    oht = din("oht", [NE, PH], BF)
    ohsel8 = din("ohsel8", [16, 128, 128], BF)   # bl hs-replication one-hots
    ohsel16 = din("ohsel16", [8, 128, 128], BF)  # bl ts-replication one-hots
    whT = din("whT", [2 * H, EMB], BF)          # head_W.T
    wtT = din("wtT", [2 * H, EMB], BF)          # tail_W.T
    wc = din("wc", [EMB * BS, NCLS], BF)        # proj_W.T @ cls_W.T
    hbb = din("hbb", [NE, EMB], F32)            # head_b broadcast to entities
    tbb = din("tbb", [NE, EMB], F32)
    clsb = din("clsb", [NCLS, 1], F32)
    out = nc.dram_tensor("out", [NCLS, PH], F32, kind="ExternalOutput").ap()

    with tile.TileContext(nc) as tc:
        with (
            nc.allow_low_precision(reason="bf16 head-plane tree reduction"),
            tc.tile_pool(name="persist", bufs=1) as pp,
            tc.tile_pool(name="work", bufs=2) as wp,
        ):
            # ---------- persistent SBUF tensors ----------
            ident = pp.tile([128, 128], F32)
            make_identity(nc, ident[:])
            # DMA order == need order: tiny metas, then the attention gather
            # (gates fused S1+S3), then everything needed from stage 4 on,
            # which streams underneath the fused compute phase.
            aidx_sb = pp.tile([128, NROWS_PAD // 16], mybir.dt.int16)
            nc.sync.dma_start(aidx_sb[:], aidx[:])
            midx_sb = pp.tile([128, MTILES], I32)
            nc.sync.dma_start(midx_sb[:], midx[:].rearrange("t p -> p t"))
            msel_sb = pp.tile([128, MTILES, NE], BF)
            nc.sync.dma_start(msel_sb[:], msel[:].rearrange("t p e -> p t e"))
            mosum_sb = pp.tile([128, NSEG, NE], BF)
            nc.sync.dma_start(mosum_sb[:], mosum[:].rearrange("t p e -> p t e"))
            ohh_sb = pp.tile([NE, PH], BF)
            nc.sync.dma_start(ohh_sb[:], ohh[:])
            oht_sb = pp.tile([NE, PH], BF)
            nc.sync.dma_start(oht_sb[:], oht[:])
            attg = pp.tile([128, NTILES, L], BF)
            nc.gpsimd.load_library(library_config.mlp)
            # 4 chunks (512/512/512/640 rows) so stage-1 matmuls can start
            # on chunk 0 while later chunks are still gathering
            for k, nk in enumerate((512, 512, 512, 640)):
                nc.gpsimd.dma_gather(
                    attg[:, k * 4:k * 4 + nk // 128, :], att[:],
                    aidx_sb[:, k * 32:k * 32 + nk // 16], nk, nk, L,
                    single_packet=False)
            # mention-row gather + exp now: ex_sb sits ready for stage 2,
            # which is deferred past the fused S1+S3 phase
            ex_sb = pp.tile([128, MTILES, H], BF)
            for c in range(MTILES):
                mg = wp.tile([128, H], F32, tag="mg", bufs=2)
                nc.gpsimd.indirect_dma_start(
                    out=mg[:], out_offset=None, in_=seq[:],
                    in_offset=bass.IndirectOffsetOnAxis(
                        ap=midx_sb[:, c:c + 1], axis=0),
                )
                nc.scalar.activation(ex_sb[:, c, :], mg[:], AF.Exp)
            # heavy loads ride the gpsimd SWDGE queue BEHIND the attention
            # gather: strict in-queue order gives the gather (which gates the
            # fused S1+S3 phase) full DMA bandwidth; these stream under compute
            seq_sb = pp.tile([128, LT, H], BF)
            nc.gpsimd.dma_start(seq_sb[:], seqb[:].rearrange("(t p) h -> p t h", p=128))
            whT_sb = pp.tile([128, 12, EMB], BF)
            nc.gpsimd.dma_start(whT_sb[:], whT[:].rearrange("(t p) o -> p t o", p=128))
            wtT_sb = pp.tile([128, 12, EMB], BF)
            nc.gpsimd.dma_start(wtT_sb[:], wtT[:].rearrange("(t p) o -> p t o", p=128))
            hbb_sb = pp.tile([NE, EMB], F32)
            nc.sync.dma_start(hbb_sb[:], hbb[:])
            tbb_sb = pp.tile([NE, EMB], F32)
            nc.sync.dma_start(tbb_sb[:], tbb[:])
            clsb_sb = pp.tile([NCLS, 1], F32)
            nc.sync.dma_start(clsb_sb[:], clsb[:])
            oh8_sb = pp.tile([128, 16, 128], BF)
            nc.gpsimd.dma_start(oh8_sb[:], ohsel8[:].rearrange("v p m -> p v m"))
            oh16_sb = pp.tile([128, 8, 128], BF)
            nc.gpsimd.dma_start(oh16_sb[:], ohsel16[:].rearrange("v p m -> p v m"))
            wc_sb = pp.tile([128, KT, NCLS], BF)
            nc.gpsimd.dma_start(wc_sb[:], wc[:].rearrange("(t p) n -> p t n", p=128))
            onecol = pp.tile([128, 1], BF)
            nc.vector.memset(onecol[:], 1.0)
            onerow = pp.tile([1, 128], BF)
            nc.vector.memset(onerow[:], 1.0)

            # e_att[nh] tiles [30, 1024] bf16 (un-normalized mention sums)
            ea = [pp.tile([NE, L], BF, tag=f"ea{nh}", name=f"ea{nh}")
                  for nh in range(NH)]
            # u tiles [128, 8, 300] bf16 (l on partitions)
            u_sb = pp.tile([128, LT, PH], BF)
            # misc small persistents
            e1h_sb = pp.tile([NE, EMB], BF, tag="e1h")
            e1t_sb = pp.tile([NE, EMB], BF, tag="e1t")
            eembT_sb = pp.tile([128, HC, NE], BF)
            rs_sb = pp.tile([128, HC, PH], BF)
            hs_sb = pp.tile([128, HC, PH], BF, tag="hs")
            ts_sb = pp.tile([128, HC, PH], BF, tag="ts")

            class _Bail(Exception):
                pass

            def bail(n):
                if stages < n:
                    nc.gpsimd.dma_start(
                        out[:], mosum_sb[0:NCLS, 0:10, 0:NE])
                    raise _Bail()

            # u accumulator: [128, lt, 2, 300] bf16, two half-planes folded
            # into u_sb at stage-4 entry
            u2_sb = pp.tile([128, LT, 2, PH], BF)

            try:
                for _rep in range(reps):
                    bail(1)
                    # ---- fused stages 1+3, software-pipelined by head pair:
                    # stage-1 segment matmuls for pair q+1 run on PE while
                    # stage-3 expansion products for pair q drain through
                    # Act (tail-side evac) and DVE (mult + accumulate).
                    # PSUM: pnh 2 banks + hp 2x2 + tp 2x1 = 8 banks. ----
                    with tc.tile_pool(name="ps_f", bufs=1, space="PSUM") as ps_f:
                        segs_of = {}
                        for si, (c, nh, first, last) in enumerate(SEGS):
                            segs_of.setdefault(nh, []).append((si, c, first, last))

                        def emit_s1_pair(q):
                            for nh in (2 * q, 2 * q + 1):
                                # shares the 2-bank slot ring with the tp tiles
                                # (disjoint lifetimes, same footprint)
                                pnh = ps_f.tile([NE, L], F32, tag="pt2", bufs=2,
                                                name=f"pnh{nh}")
                                for si, c, first, last in segs_of[nh]:
                                    for n0 in (0, 512):
                                        nc.tensor.matmul(
                                            out=pnh[:, n0:n0 + 512],
                                            lhsT=mosum_sb[:, si, :],
                                            rhs=attg[:, c, n0:n0 + 512],
                                            start=first, stop=last)
                                nc.scalar.activation(ea[nh][:], pnh[:], AF.Copy)

                        def emit_s3_pair(q):
                            for lc in range(LT):
                                hp = ps_f.tile([128, 2, 512], F32, tag="hp", bufs=2)
                                tp = ps_f.tile([128, 2, 512], F32, tag="pt2", bufs=2)
                                for j in range(2):
                                    sl = ea[q * 2 + j][:, lc * 128:(lc + 1) * 128]
                                    nc.tensor.matmul(tp[:, j, 0:PH], lhsT=sl,
                                                     rhs=oht_sb[:], start=True,
                                                     stop=True)
                                    nc.tensor.matmul(hp[:, j, 0:PH], lhsT=sl,
                                                     rhs=ohh_sb[:], start=True,
                                                     stop=True)
                                # one PSUM operand max per DVE op: evacuate
                                # the tail side on Act, multiply on DVE
                                tq = wp.tile([128, 2, PH], BF, tag="tq", bufs=3)
                                nc.scalar.activation(tq[:], tp[:, :, 0:PH], AF.Copy)
                                if q == 0:
                                    nc.vector.tensor_tensor(
                                        u2_sb[:, lc, :, :], hp[:, :, 0:PH], tq[:],
                                        op=ALU.mult)
                                else:
                                    pr = wp.tile([128, 2, PH], BF, tag="pr", bufs=3)
                                    nc.vector.tensor_tensor(
                                        pr[:], hp[:, :, 0:PH], tq[:], op=ALU.mult)
                                    # SBUF-only accumulate: odd l-tiles ride
                                    # the otherwise-idle GPSIMD engine
                                    eng = nc.gpsimd if lc % 2 else nc.vector
                                    eng.tensor_tensor(
                                        u2_sb[:, lc, :, :], u2_sb[:, lc, :, :],
                                        pr[:], op=ALU.add)
                                if q == 5:
                                    # final fold for this l-tile on GPSIMD,
                                    # inside the fused phase
                                    nc.gpsimd.tensor_tensor(
                                        u_sb[:, lc, :], u2_sb[:, lc, 0, :],
                                        u2_sb[:, lc, 1, :], op=ALU.add)

                        # s3(q) first, s1(q+1) second: the in-order PE queue
                        # serves stage-3's expansion matmuls (which gate DVE)
                        # before the next pair's segment sums, and the Act
                        # queue serves tq copies before the next ea evacs
                        emit_s1_pair(0)
                        for q in range(6):
                            emit_s3_pair(q)
                            if q + 1 < 6:
                                emit_s1_pair(q + 1)

                    bail(4)
                    # ---- stage 2 (deferred): e_emb + E1h/E1t; overlaps
                    # with the stage-4 window below ----
                    with tc.tile_pool(name="ps_sm", bufs=1, space="PSUM") as ps_sm2, \
                         tc.tile_pool(name="ps_rs", bufs=1, space="PSUM") as ps_rs:
                        psum_ee = ps_sm2.tile([NE, EMB], F32, tag="pee")
                        for c in range(MTILES):
                            nc.tensor.matmul(
                                out=psum_ee[:, 0:512],
                                lhsT=msel_sb[:, c, :], rhs=ex_sb[:, c, 0:512],
                                start=(c == 0), stop=(c == MTILES - 1))
                            nc.tensor.matmul(
                                out=psum_ee[:, 512:H],
                                lhsT=msel_sb[:, c, :], rhs=ex_sb[:, c, 512:H],
                                start=(c == 0), stop=(c == MTILES - 1))
                        eemb = wp.tile([NE, EMB], F32, tag="eemb", bufs=1)
                        # log(x + 1e-37): tiny bias keeps fully-masked entities finite
                        lneps = pp.tile([NE, 1], F32)
                        nc.vector.memset(lneps[:], 1e-37)
                        nc.scalar.activation(eemb[:], psum_ee[:], AF.Ln, bias=lneps[:])
                        for hc in range(HC):
                            pt = ps_sm2.tile([128, NE], F32, tag="ptr2", bufs=1)
                            nc.tensor.transpose(pt[:], eemb[:, hc * 128:(hc + 1) * 128],
                                                ident[0:NE, 0:NE])
                            nc.scalar.activation(eembT_sb[:, hc, :], pt[:], AF.Copy)
                        for (wT, bb, e1) in ((whT_sb, hbb_sb, e1h_sb),
                                             (wtT_sb, tbb_sb, e1t_sb)):
                            psum_e1 = ps_sm2.tile([NE, EMB], F32, tag="pee", bufs=1)
                            for ic in range(HC):
                                for n0 in (0, 512):
                                    nw = min(512, EMB - n0)
                                    nc.tensor.matmul(
                                        out=psum_e1[:, n0:n0 + nw],
                                        lhsT=eembT_sb[:, ic, :],
                                        rhs=wT[:, ic, n0:n0 + nw],
                                        start=(ic == 0), stop=(ic == HC - 1))
                            nc.vector.tensor_tensor(e1[:], psum_e1[:], bb[:], op=ALU.add)

                        # ---- stage 4: rs_T = (u @ [seq|1]) / z ----
                        psz = ps_rs.tile([1, PH], F32, tag="psz")
                        for lc in range(LT):
                            nc.tensor.matmul(psz[:], lhsT=onecol[:], rhs=u_sb[:, lc, :],
                                             start=(lc == 0), stop=(lc == LT - 1))
                        rz = wp.tile([1, PH], BF, tag="rz")
                        nc.vector.reciprocal(rz[:], psz[:])
                        pzb = ps_rs.tile([128, PH], F32, tag="pzb")
                        nc.tensor.matmul(pzb[:], lhsT=onerow[:], rhs=rz[:],
                                         start=True, stop=True)
                        zb_sb = wp.tile([128, PH], F32, tag="zb_sb", bufs=1)
                        nc.scalar.activation(zb_sb[:], pzb[:], AF.Copy)
                        for mc in range(HC):
                            prs = ps_rs.tile([128, PH], F32, tag="prs", bufs=3)
                            for lc in range(LT):
                                nc.tensor.matmul(
                                    prs[:],
                                    lhsT=seq_sb[:, lc, mc * 128:(mc + 1) * 128],
                                    rhs=u_sb[:, lc, :],
                                    start=(lc == 0), stop=(lc == LT - 1))
                            nc.vector.tensor_tensor(rs_sb[:, mc, :], prs[:], zb_sb[:],
                                                    op=ALU.mult)

                    bail(5)
                    # ---- stage 5+6 interleaved: extractors feed the block-
                    # bilinear classifier as soon as each 128-chunk is ready ----
                    with (
                        tc.tile_pool(name="ps_hs", bufs=1, space="PSUM") as ps_hs,
                        tc.tile_pool(name="ps_bl", bufs=2, space="PSUM") as ps_bl,
                        tc.tile_pool(name="ps_lg", bufs=1, space="PSUM") as ps_lg,
                    ):
                        plg = ps_lg.tile([NCLS, PH], F32)
                        for oc in range(HC):
                            # hs/ts chunk oc = tanh(W2 @ rs + E1[pair])
                            for (wT, e1, oh, dst) in ((whT_sb, e1h_sb, ohh_sb, hs_sb),
                                                      (wtT_sb, e1t_sb, oht_sb, ts_sb)):
                                ph = ps_hs.tile([128, PH], F32, tag="ph", bufs=2)
                                for ic in range(HC):
                                    nc.tensor.matmul(
                                        ph[:],
                                        lhsT=wT[:, HC + ic, oc * 128:(oc + 1) * 128],
                                        rhs=rs_sb[:, ic, :],
                                        start=(ic == 0), stop=False)
                                nc.tensor.matmul(
                                    ph[:], lhsT=e1[:, oc * 128:(oc + 1) * 128],
                                    rhs=oh[:], start=False, stop=True)
                                nc.scalar.activation(dst[:, oc, :], ph[:], AF.Tanh)

                            if stages < 6:
                                continue
                            # classifier blocks kb using hs/ts chunk oc:
                            # trep computed once per kb, reused by both halves
                            for kb in range(oc * 8, oc * 8 + 8):
                                v16 = kb % 8
                                ptr = ps_bl.tile([128, 512], F32, tag="ptr",
                                                 bufs=1)
                                nc.tensor.matmul(
                                    ptr[:, 0:PH], lhsT=oh16_sb[:, v16, :],
                                    rhs=ts_sb[:, oc, :], start=True, stop=True)
                                # trep used by both halves: evacuate once (Act)
                                trs = wp.tile([128, PH], BF, tag="trs", bufs=2)
                                nc.scalar.activation(trs[:], ptr[:, 0:PH],
                                                     AF.Copy)
                                phr = ps_bl.tile([128, 2, 512], F32, tag="phr")
                                for half in (0, 1):
                                    v8 = (kb % 8) * 2 + half
                                    nc.tensor.matmul(
                                        phr[:, half, 0:PH], lhsT=oh8_sb[:, v8, :],
                                        rhs=hs_sb[:, oc, :], start=True, stop=True)
                                blt = wp.tile([128, 2, PH], BF, tag="blt", bufs=2)
                                nc.vector.tensor_tensor(
                                    blt[:], phr[:, :, 0:PH],
                                    trs[:].unsqueeze(1).broadcast_to(
                                        (128, 2, PH)),
                                    op=ALU.mult)
                                for half in (0, 1):
                                    kt = kb * 2 + half
                                    nc.tensor.matmul(
                                        plg[:], lhsT=wc_sb[:, kt, :],
                                        rhs=blt[:, half, :],
                                        start=(kt == 0), stop=(kt == KT - 1))
                        bail(6)
                        og = wp.tile([NCLS, PH], F32, tag="og", bufs=1)
                        nc.scalar.activation(og[:], plg[:], AF.Identity,
                                             bias=clsb_sb[:])
                        nc.sync.dma_start(out[:], og[:])
            except _Bail:
                pass

    nc.compile()
    return nc


_PROG = None
TRACE = False          # set True (e.g. from test.py) to profile the run
LAST_RES = None        # BassKernelResults of the last kernel() call


def _host_prep(inputs):
    """Build the 8 per-core input maps from full inputs."""
    seqf = np.asarray(inputs["sequence_output"], np.float32)
    attf = np.asarray(inputs["attention"], np.float32)
    mask = np.asarray(inputs["mention_mask"], np.float32)
    midx_full = np.asarray(inputs["mention_idx"]).astype(np.int64)
    hts = np.asarray(inputs["hts"]).astype(np.int64)
    head_W = np.asarray(inputs["head_W"], np.float32)
    tail_W = np.asarray(inputs["tail_W"], np.float32)
    head_b = np.asarray(inputs["head_b"], np.float32)
    tail_b = np.asarray(inputs["tail_b"], np.float32)
    proj_W = np.asarray(inputs["proj_W"], np.float32)
    cls_W = np.asarray(inputs["cls_W"], np.float32)
    cls_b = np.asarray(inputs["cls_b"], np.float32)

    wc = (proj_W.T @ cls_W.T).astype(BF16)                     # [12288, 97]
    whT = np.ascontiguousarray(head_W.T).astype(BF16)          # [1536, 768]
    wtT = np.ascontiguousarray(tail_W.T).astype(BF16)
    hbb = np.broadcast_to(head_b, (NE, EMB)).astype(np.float32).copy()
    tbb = np.broadcast_to(tail_b, (NE, EMB)).astype(np.float32).copy()
    clsb = cls_b.reshape(NCLS, 1).copy()
    pp_, mm_ = np.meshgrid(np.arange(128), np.arange(128), indexing="ij")
    ohsel8 = np.stack([(pp_ == 8 * v + mm_ // 16) for v in range(16)]
                      ).astype(BF16)
    ohsel16 = np.stack([(pp_ == 16 * v + mm_ % 16) for v in range(8)]
                       ).astype(BF16)

    in_maps = []
    for c in range(NCORES):
        b, half = c // 2, c % 2
        p0 = half * PH
        # attention-row gather ids: r = nh*180 + e*6 + m -> nh*1024 + idx
        rows = (np.arange(NH)[:, None] * L +
                midx_full[b].reshape(1, -1)).reshape(-1)       # [2160]
        aidx = np.zeros(NROWS_PAD, np.int16)
        aidx[:NROWS] = rows.astype(np.int16)
        aidx16 = np.tile(aidx.reshape(-1, 16).T, (8, 1)).astype(np.int16)
        # mosum: per-segment [128, NE] with mask value at entity column,
        # rows outside the segment's head zeroed
        ecol = np.tile(np.repeat(np.arange(NE), M), NH)        # [2160]
        mval = np.tile(mask[b].reshape(-1), NH)                # [2160]
        dense = np.zeros((NROWS_PAD, NE), np.float32)
        dense[np.arange(NROWS), ecol] = mval
        rownh = np.full(NROWS_PAD, -1, np.int64)
        rownh[:NROWS] = np.arange(NROWS) // (NE * M)
        mosum = np.zeros((NSEG, 128, NE), BF16)
        for si, (ci, nh, _f, _l) in enumerate(SEGS):
            rws = slice(ci * 128, ci * 128 + 128)
            mosum[si] = dense[rws] * (rownh[rws] == nh)[:, None]
        # mention seq gather
        midx = np.zeros(MTILES * 128, np.int32)
        midx[:MROWS] = midx_full[b].reshape(-1)
        msel = np.zeros((MTILES * 128, NE), BF16)
        msel[np.arange(MROWS), np.repeat(np.arange(NE), M)] = \
            (mask[b].reshape(-1) > 0).astype(np.float32)
        # pair one-hots
        hh = hts[b, p0:p0 + PH, 0]
        tt = hts[b, p0:p0 + PH, 1]
        ohh = np.zeros((NE, PH), BF16)
        ohh[hh, np.arange(PH)] = 1
        oht = np.zeros((NE, PH), BF16)
        oht[tt, np.arange(PH)] = 1
        in_maps.append({
            "att": attf[b].reshape(NH * L, L).astype(BF16),
            "seq": seqf[b],
            "seqb": seqf[b].astype(BF16),
            "aidx": aidx16,
            "midx": midx.reshape(MTILES, 128),
            "mosum": mosum,
            "msel": msel.reshape(MTILES, 128, NE),
            "ohh": ohh, "oht": oht, "ohsel8": ohsel8, "ohsel16": ohsel16,
            "whT": whT, "wtT": wtT, "wc": wc,
            "hbb": hbb, "tbb": tbb, "clsb": clsb,
        })
    return in_maps


def kernel(**inputs):
    global _PROG, LAST_RES
    if _PROG is None:
        _PROG = build_program()
    in_maps = _host_prep(inputs)
    res = bass_utils.run_bass_kernel_spmd(
        _PROG, in_maps, core_ids=list(range(NCORES)), trace=TRACE)
    LAST_RES = res
    logits = np.zeros((B, P, NCLS), np.float32)
    for c in range(NCORES):
        b, half = c // 2, c % 2
        logits[b, half * PH:(half + 1) * PH, :] = res.results[c]["out"].T
    return logits
